# revision 2
# baseline (speedup 1.0000x reference)
"""Multi-head attention (no-transpose head reshape) on 8 trn2 cores — v3.

Sharding: core c gets b=c//4, heads 4*(c%4)..+4 (rows 512*(c%4)..+512 of
batch b). 32 independent (b, h) tasks, no collectives.

v3 vs baseline (219.7us -> ~210us):
  - Row-tiled score pairs: the two N=512 score MMs of each k-group run
    CONCURRENTLY in the PE array (tile_position (0,0)+(64,0), K=64
    each), halving scores PE time. Needs partition-swapped copies of
    both XqT (XqTs) and XkT (XkTs), made by gpsimd SBUF-SBUF DMA.
  - Unit = (head, m-half); m-half outer so ONE [65,1024] AV accumulator
    (2 PSUM banks) is live. Scores live in R01 [128,2048] (4 banks) +
    R2 [128,1024] (2 banks): superstep = 3 k-groups (pos0/pos1 in R01,
    pos2 in R2); exp batched N=2048 over R01, N=1024 over R2. Separate
    tiles matter: Tile tracks cross-engine PSUM deps at TILE
    granularity, so pairC/V-borrows in R2 don't serialize against the
    R01 exp (and vice versa).
  - PE emission order per superstep keeps ACT gapless: pairs first
    (they only wait the previous exp read), then deferred AVs, then bg.
  - Input DMA: host pre-lays x as [p, t, m] and W as [p, s, t, c]
    (fully contiguous per partition -> ~128 descriptors per DMA instead
    of 1024+), weights in per-slab DMAs so projection chains pipeline
    behind the transfers. PE warm-up MMs gated on the xq DMA so HAM
    reaches K=8/8 before the chains.
  - Pre-attention: full Q/K projections as N=512 chains alternating
    R01/R2 slots (tile-granular WAR => no chain-boundary stalls);
    V dripped per head through the bg queue into R2 between supersteps.
  - Finish per unit: ONE whole-unit DMA-transpose (HWDGE xbar)
    [96,1024] bf16 -> [128,8,96], then rcp + per-chunk scalar-mul into
    stage, one 512KB DMA per head. No PE or PSUM involvement.
"""

import numpy as np

B, S, D, NH = 2, 2048, 1024, 16
DH = 64
NCORES = 8
HPC = NH * B // NCORES      # heads per core = 4
ROWS = HPC * (S // NH)      # projection rows per core = 512
PO = D // 128               # 8 din/dout tiles

_BUILT = {}


def _build_nc(reps=1, salt=0.0):
    if ("nc", reps, salt) in _BUILT:
        return _BUILT[("nc", reps, salt)]

    import concourse.bass as bass
    import concourse.bacc as bacc
    import concourse.tile as tile
    from concourse import mybir
    from concourse.masks import make_identity
    from contextlib import ExitStack

    f32 = mybir.dt.float32
    bf16 = mybir.dt.bfloat16
    Exp = mybir.ActivationFunctionType.Exp

    nc = bacc.Bacc("TRN2", target_bir_lowering=False, debug=False)

    # x inputs host-laid-out as [p, t, m] (contiguous 8KB/partition)
    qT = nc.dram_tensor("qT", [128, PO, ROWS], bf16, kind="ExternalInput")
    kT = nc.dram_tensor("kT", [128, PO, ROWS], bf16, kind="ExternalInput")
    vT = nc.dram_tensor("vT", [128, PO, ROWS], bf16, kind="ExternalInput")
    # partition-major: [p, s, t, c] = W[t*128+p, s*128+c] — 16KB
    # contiguous per partition so one DMA with minimal descriptors moves
    # a whole weight matrix.
    Wq = nc.dram_tensor("Wq", [128, PO, PO, 128], bf16, kind="ExternalInput")
    Wk = nc.dram_tensor("Wk", [128, PO, PO, 128], bf16, kind="ExternalInput")
    Wv = nc.dram_tensor("Wv", [128, PO, PO, 128], bf16, kind="ExternalInput")
    bq = nc.dram_tensor("bq", [128, PO], f32, kind="ExternalInput")
    bk = nc.dram_tensor("bk", [128, PO], f32, kind="ExternalInput")
    bv = nc.dram_tensor("bv", [1, D], f32, kind="ExternalInput")
    out = nc.dram_tensor("out", [HPC, S, DH], f32, kind="ExternalOutput")
    out_w = out.ap().rearrange("h (a r) d -> h a r d", r=16)

    P = 128
    KB = S // P                # 16 k-groups per head

    with tile.TileContext(nc) as tc, ExitStack() as ctx:
        consts = ctx.enter_context(tc.tile_pool(name="consts", bufs=1))
        wrows = ctx.enter_context(tc.tile_pool(name="wrows", bufs=1))
        inputs = ctx.enter_context(tc.tile_pool(name="inputs", bufs=1))
        proj = ctx.enter_context(tc.tile_pool(name="proj", bufs=1))

        for _rep in range(reps):
            xqT = inputs.tile([P, PO, ROWS], bf16, tag="xqT")
            xkT = inputs.tile([P, PO, ROWS], bf16, tag="xkT")
            xvT = inputs.tile([P, PO, ROWS], bf16, tag="xvT")
            XqT = proj.tile([P, PO, ROWS], bf16, tag="XqT")
            XqTs = proj.tile([P, PO, ROWS], bf16, tag="XqTs")
            XkT = proj.tile([P, PO, ROWS], bf16, tag="XkT")
            XkTs = proj.tile([P, PO, ROWS], bf16, tag="XkTs")
            av_lhs = proj.tile([P, HPC, KB, DH + 1], bf16, tag="av_lhs")

            # weight layout [p, s(dout-slab), t(din-tile), c]: lhsT for
            # (dint, po) = w[:, po, dint, :]
            wq_w = wrows.tile([P, PO, PO, P], bf16, tag="wq")
            wk_w = wrows.tile([P, PO, PO, P], bf16, tag="wk")
            wv_w = wrows.tile([P, PO, PO, P], bf16, tag="wv")

            bq_sb = consts.tile([P, PO], f32, tag="bq")
            bk_sb = consts.tile([P, PO], f32, tag="bk")
            bv_sb = consts.tile([P, D], f32, tag="bv")
            identf = consts.tile([P, P], f32, tag="identf")

            # ---------------- input DMAs (deadline order) ----------------
            xqsrc = qT.ap()
            xksrc = kT.ap()
            xvsrc = vT.ap()

            # sync: Q path (attention-start critical); per-slab weight
            # DMAs (contiguous (t,c) 2KB per partition -> cheap triggers)
            # so projection chains pipeline behind the transfers.
            nc.sync.dma_start(out=bq_sb[:], in_=bq.ap())
            nc.sync.dma_start(out=bk_sb[:], in_=bk.ap())
            nc.sync.dma_start(out=xqT[:], in_=xqsrc[:])
            for s in range(PO):
                nc.sync.dma_start(out=wq_w[:, s], in_=Wq.ap()[:, s])
            # scalar: K path, then V path
            nc.scalar.dma_start(out=xkT[:], in_=xksrc[:])
            for s in range(PO):
                nc.scalar.dma_start(out=wk_w[:, s], in_=Wk.ap()[:, s])
            nc.scalar.dma_start(out=xvT[:], in_=xvsrc[:])
            nc.scalar.dma_start(out=wv_w[:, 0:4], in_=Wv.ap()[:, 0:4])
            nc.scalar.dma_start(out=wv_w[:, 4:8], in_=Wv.ap()[:, 4:8])
            # gpsimd: bias broadcast (+ swap DMAs emitted inline later)
            bv_ap = bv.ap()
            bv_bcast = bass.AP(tensor=bv_ap.tensor, offset=bv_ap.offset,
                               ap=[[0, P], [1, D]])
            nc.gpsimd.dma_start(out=bv_sb[:], in_=bv_bcast)

            make_identity(nc, identf[:])
            warm = consts.tile([1, 1], f32, tag="warm")
            nc.vector.memset(warm[:], salt)
            nc.scalar.activation(warm[:], warm[:], Exp, scale=1.0)
            nc.vector.memset(av_lhs[:, :, :, DH:DH + 1], 1.0)

            # ---------------- PSUM pools ----------------
            with tc.tile_pool(name="Rp", bufs=1, space="PSUM") as Rp, \
                 tc.tile_pool(name="avp", bufs=1, space="PSUM") as avp, \
                 tc.tile_pool(name="attn", bufs=6) as attn_pool, \
                 tc.tile_pool(name="fin", bufs=4) as fin_pool:

                # Two score tiles: cross-engine deps are tracked at TILE
                # granularity, so the batched exp reads exactly R01 and
                # pos2 work (pairC, V/tp borrows) lives in R2.
                R01 = Rp.tile([P, 2048], f32, tag="R01")
                R2 = Rp.tile([P, 1024], f32, tag="R2")

                # ---- pre-attention: FULL Q/K projections (N=512 chains,
                # one per po), cycling R slots of 512 cols. K po0 first
                # (first scores pair needs it + its swap), then Q (all po
                # gate the first pair's rhs), then K rest.
                def qk_chain(w_sb, x_sb, X_dst, X_swp, b_sb, po, ps):
                    for dint in range(PO):
                        nc.tensor.matmul(
                            ps, w_sb[:, po, dint, :],
                            x_sb[:, dint, :],
                            start=(dint == 0), stop=(dint == PO - 1))
                    nc.vector.tensor_scalar_add(X_dst[:, po, :], ps,
                                                b_sb[:, po:po + 1])
                    nc.gpsimd.dma_start(out=X_swp[0:64, po, :],
                                        in_=X_dst[64:128, po, :])
                    nc.gpsimd.dma_start(out=X_swp[64:128, po, :],
                                        in_=X_dst[0:64, po, :])

                # PE warm-up: ~1.3us of tiny matmuls gated on the same
                # deps as chain 0 (xq + Wq slab 0) so HAM reaches K=8/8
                # right as the real chains start.
                for dint in range(PO):
                    nc.tensor.matmul(R01[:, 0:P], wq_w[:, 0, dint, :],
                                     xqT[:, dint, 0:P],
                                     start=True, stop=True)

                def chain_slot(i):
                    # alternate tiles so chain i+1 never waits chain i's
                    # eviction (tile-granular WAR)
                    if i % 2 == 0:
                        return R01[:, ((i // 2) % 4) * 512:
                                   ((i // 2) % 4) * 512 + 512]
                    return R2[:, ((i // 2) % 2) * 512:
                              ((i // 2) % 2) * 512 + 512]

                def v_chunk(h, qd, ps):
                    for dint in range(PO):
                        nc.tensor.matmul(
                            ps, xvT[:, dint, h * P:(h + 1) * P],
                            wv_w[:, 2 * qd:2 * qd + 2, dint, :],
                            start=(dint == 0), stop=(dint == PO - 1))
                    nc.vector.tensor_add(
                        av_lhs[:, h, qd * 4:(qd + 1) * 4, 0:DH], ps,
                        bv_sb[:, qd * 256:(qd + 1) * 256])

                chains = ([("k", 0)] + [("q", po) for po in range(PO)] +
                          [("k", po) for po in range(1, PO)])
                for i, (which, po) in enumerate(chains):
                    if which == "q":
                        qk_chain(wq_w, xqT, XqT, XqTs, bq_sb, po,
                                 chain_slot(i))
                    else:
                        qk_chain(wk_w, xkT, XkT, XkTs, bk_sb, po,
                                 chain_slot(i))

                # ---- deferred-work queues ----
                av_queue = []     # per-k-group AV closures (order = accum order)
                bg_queue = []     # (cost_ns, is_borrow, closure)

                def drain_av(n=1):
                    for _ in range(n):
                        if av_queue:
                            av_queue.pop(0)()

                def drain_bg(budget):
                    borrowed = False
                    while bg_queue and budget > 0:
                        cost, kind, fn = bg_queue[0]
                        borrow = kind == "v"
                        if borrow and borrowed:
                            break
                        if cost > budget and budget < 1400:
                            break
                        bg_queue.pop(0)
                        fn()
                        budget -= cost
                        borrowed = borrowed or borrow

                # V projection for head h, dout-quarter qd (N=256),
                # borrowing R2 between supersteps
                def v_item(h, qd):
                    def run():
                        v_chunk(h, qd, R2[:, 0:256])
                    return (1400, "v", run)

                for qd in range(4):
                    bg_queue.append(v_item(0, qd))

                finish_state = {}

                # finish items for unit (h, mh): DMA-transpose each
                # [96,128] chunk of oT (bf16) into fin_sb [128, 96], then
                # rcp + mul on DVE. No PE or PSUM involvement at all.
                def fin_item(h, mh, j):
                    def run():
                        oT = finish_state[(h, mh)]
                        fsb = finish_state[(h, mh, "fsb")]
                        stage = finish_state.get((h, "stage"))
                        if stage is None:
                            stage = fin_pool.tile([P, 16, DH], f32,
                                                  tag="stage", bufs=2,
                                                  name=f"stage{h}")
                            finish_state[(h, "stage")] = stage
                        if j == 0:
                            # one whole-unit transpose: [96,1024]->[128,8,96]
                            nc.sync.dma_start(out=fsb[:], in_=oT[:],
                                              transpose=True)
                        rcp = fin_pool.tile([P, 1], f32, tag="rcp")
                        nc.vector.reciprocal(rcp[:], fsb[:, j, DH:DH + 1])
                        nc.vector.tensor_scalar_mul(stage[:, 2 * j + mh, :],
                                                    fsb[:, j, 0:DH], rcp[:])
                    return (150, "fin", run)

                def dma_out_item(h):
                    def run():
                        stage = finish_state.pop((h, "stage"))
                        nc.sync.dma_start(out=out_w[h, :, :, :], in_=stage[:])
                    return (100, "out", run)

                # ---------------- attention units ----------------
                def unit(h, mh, first_unit, last_unit):
                    av_t = avp.tile([DH + 1, 1024], f32, tag="av",
                                    name=f"av{h}_{mh}")
                    qA = XqT if mh == 0 else XqTs
                    qB = XqTs if mh == 0 else XqT

                    def pair(g, pos):
                        kA = XkT if (g % 2) == 0 else XkTs
                        kB = XkTs if (g % 2) == 0 else XkT
                        dst = (R01[:, 0:1024] if pos == 0 else
                               R01[:, 1024:2048] if pos == 1 else R2[:])
                        nc.tensor.matmul(
                            dst[:, 0:512],
                            kA[0:64, g // 2, h * P:(h + 1) * P],
                            qA[0:64, 0:4, h * P:(h + 1) * P],
                            start=True, stop=True)
                        nc.tensor.matmul(
                            dst[:, 512:1024],
                            kB[64:128, g // 2, h * P:(h + 1) * P],
                            qB[64:128, 4:8, h * P:(h + 1) * P],
                            start=True, stop=True)

                    def push_av(g, at_ap, first, last):
                        def run():
                            for half in range(2):
                                nc.tensor.matmul(
                                    av_t[:, half * 512:(half + 1) * 512],
                                    av_lhs[:, h, g, :],
                                    at_ap[:, half * 512:(half + 1) * 512],
                                    start=first, stop=last)
                        av_queue.append(run)

                    # PE order per superstep: pairs FIRST (they only wait
                    # the previous exp's read, so they run while ACT is on
                    # the previous pos2 exp), then deferred AV of the
                    # previous superstep, then bg, then this superstep's
                    # A/B AVs. Keeps ACT gapless.
                    hold = 3 if first_unit else 0
                    for c in range(6):
                        gA, gB, gC = 3 * c, 3 * c + 1, 3 * c + 2
                        if c < 5:
                            pair(gA, 0)
                            pair(gB, 1)
                            at2 = attn_pool.tile([P, 2048], bf16, tag="at2")
                            nc.scalar.activation(at2[:], R01[:], Exp,
                                                 scale=1.0)
                            if c >= hold:
                                drain_av(4 if first_unit else 2)
                            pair(gC, 2)
                            at1 = attn_pool.tile([P, 1024], bf16, tag="at1")
                            nc.scalar.activation(at1[:], R2[:], Exp,
                                                 scale=1.0)
                            if c >= hold:
                                drain_av(2 if first_unit else 1)
                            drain_bg(2400 if first_unit else
                                     (1900 if c % 2 == 0 else 1100))
                            push_av(gA, at2[:, 0:1024], gA == 0, False)
                            push_av(gB, at2[:, 1024:2048], False, False)
                            push_av(gC, at1[:], False, gC == KB - 1)
                        else:
                            g15 = 15
                            pair(g15, 2)
                            at1 = attn_pool.tile([P, 1024], bf16, tag="at1")
                            nc.scalar.activation(at1[:], R2[:], Exp,
                                                 scale=1.0)
                            drain_av(3)
                            drain_bg(900)
                            push_av(g15, at1[:], False, True)
                            drain_av(len(av_queue))

                    # evict accumulator (bf16, rows 0:65 of a 96-row
                    # tile so DMA-transpose chunks are [96,128]); queue
                    # finish work
                    oT = fin_pool.tile([96, 1024], bf16, tag="oT", bufs=3)
                    nc.vector.tensor_copy(oT[0:DH + 1, :], av_t[:])
                    finish_state[(h, mh)] = oT
                    fsb_t = fin_pool.tile([P, PO, 96], bf16, tag="fsb",
                                          bufs=3, name=f"fsb{h}_{mh}")
                    finish_state[(h, mh, "fsb")] = fsb_t
                    if mh == 1:
                        # stage DMA needs ALL 16 fin muls done first: flush
                        # any straggler fins of (h, 0) now, put (h, 1) fins
                        # in front, and the dma_out at the very back.
                        rest = []
                        for item in bg_queue:
                            if item[1] == "fin":
                                item[2]()
                            else:
                                rest.append(item)
                        bg_queue[:] = rest
                    bg_queue[0:0] = [fin_item(h, mh, j) for j in range(PO)]
                    if mh == 1:
                        bg_queue.append(dma_out_item(h))

                # unit loop with V drip for upcoming heads
                for h in range(HPC):
                    for mh in range(2):
                        if mh == 1 and h + 1 < HPC:
                            # front: next head's AVs need these within ~1 unit
                            bg_queue[0:0] = [v_item(h + 1, qd)
                                             for qd in range(4)]
                        unit(h, mh, first_unit=(h == 0 and mh == 0),
                             last_unit=(h == HPC - 1 and mh == 1))

                # tail: drain remaining finish work
                while bg_queue:
                    _, _, fn = bg_queue.pop(0)
                    fn()

    nc.compile()
    _dedupe_ldweights(nc)
    _BUILT[("nc", reps, salt)] = nc
    return nc


def _dedupe_ldweights(nc):
    """Remove InstLdweights that reload the stationary already resident in
    the PE array (consecutive matmuls sharing lhsT)."""
    def key(a):
        return (str(a.memref), a.offset, str(a.ap), str(a.dtype))

    for f in nc.m.functions:
        for b in f.blocks:
            last = None
            keep = []
            for i in b.instructions:
                tn = type(i).__name__
                if tn == "InstLdweights":
                    k = key(i.ins[0])
                    si = i.sync_info
                    clean = (si is None) or (not si.on_wait and not si.on_update)
                    if last == k and clean:
                        continue
                    last = k
                elif tn == "InstMatmult":
                    if i.is_transpose:
                        last = None
                elif tn in ("InstDrain", "InstUnconditionalBranch", "InstCall"):
                    last = None
                keep.append(i)
            b.instructions[:] = keep


def _make_in_maps(q, k, v, Wq, bq, Wk, bk, Wv, bv):
    import ml_dtypes
    bfl = ml_dtypes.bfloat16

    q = np.asarray(q, dtype=np.float32)
    k = np.asarray(k, dtype=np.float32)
    v = np.asarray(v, dtype=np.float32)

    def slab(w):
        # [din, dout] -> [p, s, t, c] with din = t*128+p, dout = s*128+c
        return np.ascontiguousarray(
            w.reshape(PO, 128, PO, 128).transpose(1, 2, 0, 3))

    Wq_b = slab((np.asarray(Wq, np.float32) * 0.125).astype(bfl))
    Wk_b = slab(np.asarray(Wk, np.float32).astype(bfl))
    Wv_b = slab(np.asarray(Wv, np.float32).astype(bfl))
    bq_t = np.ascontiguousarray(
        (np.asarray(bq, np.float32) * 0.125).reshape(PO, 128).T)
    bk_t = np.ascontiguousarray(np.asarray(bk, np.float32).reshape(PO, 128).T)
    bv_t = np.ascontiguousarray(np.asarray(bv, np.float32).reshape(1, D))

    in_maps = []
    for c in range(NCORES):
        b = c // (NCORES // B)
        r0 = (c % (NCORES // B)) * ROWS
        def xprep(x):
            # [ROWS, D] -> [p, t, m] with din = t*128 + p
            return np.ascontiguousarray(
                x[b, r0:r0 + ROWS, :].T.astype(bfl)
                .reshape(PO, 128, ROWS).transpose(1, 0, 2))
        in_maps.append({
            "qT": xprep(q), "kT": xprep(k), "vT": xprep(v),
            "Wq": Wq_b, "Wk": Wk_b, "Wv": Wv_b,
            "bq": bq_t, "bk": bk_t, "bv": bv_t,
        })
    return in_maps


def kernel(q, k, v, Wq, bq, Wk, bk, Wv, bv):
    from concourse.bass_utils import run_bass_kernel_spmd

    nc = _build_nc()
    in_maps = _make_in_maps(q, k, v, Wq, bq, Wk, bk, Wv, bv)
    res = run_bass_kernel_spmd(nc, in_maps, core_ids=list(range(NCORES)))

    outp = np.empty((B, NH, S, DH), dtype=np.float32)
    for c in range(NCORES):
        b = c // (NCORES // B)
        h0 = (c % (NCORES // B)) * HPC
        outp[b, h0:h0 + HPC] = res.results[c]["out"]
    return outp


# revision 4
# speedup vs baseline: 1.0413x; 1.0413x over previous
"""Multi-head attention (no-transpose head reshape) on 8 trn2 cores — v3.

Sharding: core c gets b=c//4, heads 4*(c%4)..+4 (rows 512*(c%4)..+512 of
batch b). 32 independent (b, h) tasks, no collectives.

v3 vs baseline (219.7us -> ~210us):
  - Row-tiled score pairs: the two N=512 score MMs of each k-group run
    CONCURRENTLY in the PE array (tile_position (0,0)+(64,0), K=64
    each), halving scores PE time. Needs partition-swapped copies of
    both XqT (XqTs) and XkT (XkTs), made by gpsimd SBUF-SBUF DMA.
  - Unit = (head, m-half); m-half outer so ONE [65,1024] AV accumulator
    (2 PSUM banks) is live. Scores live in R01 [128,2048] (4 banks) +
    R2 [128,1024] (2 banks): superstep = 3 k-groups (pos0/pos1 in R01,
    pos2 in R2); exp batched N=2048 over R01, N=1024 over R2. Separate
    tiles matter: Tile tracks cross-engine PSUM deps at TILE
    granularity, so pairC/V-borrows in R2 don't serialize against the
    R01 exp (and vice versa).
  - PE emission order per superstep keeps ACT gapless: pairs first
    (they only wait the previous exp read), then deferred AVs, then bg.
  - Input DMA: host pre-lays x as [p, t, m] and W as [p, s, t, c]
    (fully contiguous per partition -> ~128 descriptors per DMA instead
    of 1024+), weights in per-slab DMAs so projection chains pipeline
    behind the transfers. PE warm-up MMs gated on the xq DMA so HAM
    reaches K=8/8 before the chains.
  - Pre-attention: full Q/K projections as N=512 chains alternating
    R01/R2 slots (tile-granular WAR => no chain-boundary stalls);
    V dripped per head through the bg queue into R2 between supersteps.
  - Finish per unit: ONE whole-unit DMA-transpose (HWDGE xbar)
    [96,1024] bf16 -> [128,8,96], then rcp + per-chunk scalar-mul into
    stage, one 512KB DMA per head. No PE or PSUM involvement.
"""

import numpy as np

B, S, D, NH = 2, 2048, 1024, 16
DH = 64
NCORES = 8
HPC = NH * B // NCORES      # heads per core = 4
ROWS = HPC * (S // NH)      # projection rows per core = 512
PO = D // 128               # 8 din/dout tiles

_BUILT = {}


def _build_nc(reps=1, salt=0.0):
    if ("nc", reps, salt) in _BUILT:
        return _BUILT[("nc", reps, salt)]

    import concourse.bass as bass
    import concourse.bacc as bacc
    import concourse.tile as tile
    from concourse import mybir
    from concourse.masks import make_identity
    from contextlib import ExitStack

    f32 = mybir.dt.float32
    bf16 = mybir.dt.bfloat16
    Exp = mybir.ActivationFunctionType.Exp

    nc = bacc.Bacc("TRN2", target_bir_lowering=False, debug=False)

    # x inputs host-laid-out as [p, t, m] (contiguous 8KB/partition)
    qT = nc.dram_tensor("qT", [128, PO, ROWS], bf16, kind="ExternalInput")
    kT = nc.dram_tensor("kT", [128, PO, ROWS], bf16, kind="ExternalInput")
    vT = nc.dram_tensor("vT", [128, PO, ROWS], bf16, kind="ExternalInput")
    # partition-major: [p, s, t, c] = W[t*128+p, s*128+c] — 16KB
    # contiguous per partition so one DMA with minimal descriptors moves
    # a whole weight matrix.
    Wq = nc.dram_tensor("Wq", [128, PO, PO, 128], bf16, kind="ExternalInput")
    Wk = nc.dram_tensor("Wk", [128, PO, PO, 128], bf16, kind="ExternalInput")
    Wv = nc.dram_tensor("Wv", [128, PO, PO, 128], bf16, kind="ExternalInput")
    bq = nc.dram_tensor("bq", [128, PO], f32, kind="ExternalInput")
    bk = nc.dram_tensor("bk", [128, PO], f32, kind="ExternalInput")
    bv = nc.dram_tensor("bv", [1, D], f32, kind="ExternalInput")
    out = nc.dram_tensor("out", [HPC, S, DH], f32, kind="ExternalOutput")
    out_w = out.ap().rearrange("h (a r) d -> h a r d", r=16)

    P = 128
    KB = S // P                # 16 k-groups per head

    with tile.TileContext(nc) as tc, ExitStack() as ctx:
        consts = ctx.enter_context(tc.tile_pool(name="consts", bufs=1))
        wrows = ctx.enter_context(tc.tile_pool(name="wrows", bufs=1))
        inputs = ctx.enter_context(tc.tile_pool(name="inputs", bufs=1))
        proj = ctx.enter_context(tc.tile_pool(name="proj", bufs=1))

        for _rep in range(reps):
            xqT = inputs.tile([P, PO, ROWS], bf16, tag="xqT")
            xkT = inputs.tile([P, PO, ROWS], bf16, tag="xkT")
            xvT = inputs.tile([P, PO, ROWS], bf16, tag="xvT")
            XqT = proj.tile([P, PO, ROWS], bf16, tag="XqT")
            XqTs = proj.tile([P, PO, ROWS], bf16, tag="XqTs")
            XkT = proj.tile([P, PO, ROWS], bf16, tag="XkT")
            XkTs = proj.tile([P, PO, ROWS], bf16, tag="XkTs")
            av_lhs = proj.tile([P, HPC, KB, DH + 1], bf16, tag="av_lhs")

            # weight layout [p, s(dout-slab), t(din-tile), c]: lhsT for
            # (dint, po) = w[:, po, dint, :]
            wq_w = wrows.tile([P, PO, PO, P], bf16, tag="wq")
            wk_w = wrows.tile([P, PO, PO, P], bf16, tag="wk")
            wv_w = wrows.tile([P, PO, PO, P], bf16, tag="wv")

            bq_sb = consts.tile([P, PO], f32, tag="bq")
            bk_sb = consts.tile([P, PO], f32, tag="bk")
            bv_sb = consts.tile([P, D], f32, tag="bv")
            identf = consts.tile([P, P], f32, tag="identf")

            # ---------------- input DMAs (deadline order) ----------------
            xqsrc = qT.ap()
            xksrc = kT.ap()
            xvsrc = vT.ap()

            # sync: Q path (attention-start critical); per-slab weight
            # DMAs (contiguous (t,c) 2KB per partition -> cheap triggers)
            # so projection chains pipeline behind the transfers.
            # exp table load first, before any scalar-queue DMA triggers
            warm = consts.tile([1, 1], f32, tag="warm")
            nc.vector.memset(warm[:], salt)
            nc.scalar.activation(warm[:], warm[:], Exp, scale=1.0)
            nc.sync.dma_start(out=bq_sb[:], in_=bq.ap())
            nc.sync.dma_start(out=bk_sb[:], in_=bk.ap())
            nc.sync.dma_start(out=xqT[:], in_=xqsrc[:])
            for s in range(PO):
                nc.sync.dma_start(out=wq_w[:, s], in_=Wq.ap()[:, s])
            # scalar: K path, then V path
            nc.scalar.dma_start(out=xkT[:], in_=xksrc[:])
            for s in range(PO):
                nc.scalar.dma_start(out=wk_w[:, s], in_=Wk.ap()[:, s])
            nc.scalar.dma_start(out=xvT[:], in_=xvsrc[:])
            nc.scalar.dma_start(out=wv_w[:, 0:4], in_=Wv.ap()[:, 0:4])
            nc.scalar.dma_start(out=wv_w[:, 4:8], in_=Wv.ap()[:, 4:8])
            # gpsimd: bias broadcast (+ swap DMAs emitted inline later)
            bv_ap = bv.ap()
            bv_bcast = bass.AP(tensor=bv_ap.tensor, offset=bv_ap.offset,
                               ap=[[0, P], [1, D]])
            nc.gpsimd.dma_start(out=bv_sb[:], in_=bv_bcast)

            make_identity(nc, identf[:])
            nc.vector.memset(av_lhs[:, :, :, DH:DH + 1], 1.0)

            # ---------------- PSUM pools ----------------
            with tc.tile_pool(name="Rp", bufs=1, space="PSUM") as Rp, \
                 tc.tile_pool(name="avp", bufs=1, space="PSUM") as avp, \
                 tc.tile_pool(name="attn", bufs=6) as attn_pool, \
                 tc.tile_pool(name="fin", bufs=4) as fin_pool:

                # Two score tiles: cross-engine deps are tracked at TILE
                # granularity, so the batched exp reads exactly R01 and
                # pos2 work (pairC, V/tp borrows) lives in R2.
                R01 = Rp.tile([P, 2048], f32, tag="R01")
                R2 = Rp.tile([P, 1024], f32, tag="R2")

                # ---- pre-attention: FULL Q/K projections (N=512 chains,
                # one per po), cycling R slots of 512 cols. K po0 first
                # (first scores pair needs it + its swap), then Q (all po
                # gate the first pair's rhs), then K rest.
                def qk_chain(w_sb, x_sb, X_dst, X_swp, b_sb, po, ps):
                    for dint in range(PO):
                        nc.tensor.matmul(
                            ps, w_sb[:, po, dint, :],
                            x_sb[:, dint, :],
                            start=(dint == 0), stop=(dint == PO - 1))
                    nc.vector.tensor_scalar_add(X_dst[:, po, :], ps,
                                                b_sb[:, po:po + 1])
                    nc.gpsimd.dma_start(out=X_swp[0:64, po, :],
                                        in_=X_dst[64:128, po, :])
                    nc.gpsimd.dma_start(out=X_swp[64:128, po, :],
                                        in_=X_dst[0:64, po, :])

                # PE warm-up: ~1.3us of tiny matmuls gated on the same
                # deps as chain 0 (xq + Wq slab 0) so HAM reaches K=8/8
                # right as the real chains start.
                for dint in range(PO):
                    nc.tensor.matmul(R01[:, 0:P], wq_w[:, 0, dint, :],
                                     xqT[:, dint, 0:P],
                                     start=True, stop=True)

                def chain_slot(i):
                    # alternate tiles so chain i+1 never waits chain i's
                    # eviction (tile-granular WAR)
                    if i % 2 == 0:
                        return R01[:, ((i // 2) % 4) * 512:
                                   ((i // 2) % 4) * 512 + 512]
                    return R2[:, ((i // 2) % 2) * 512:
                              ((i // 2) % 2) * 512 + 512]

                def v_chunk(h, qd, ps):
                    for dint in range(PO):
                        nc.tensor.matmul(
                            ps, xvT[:, dint, h * P:(h + 1) * P],
                            wv_w[:, 2 * qd:2 * qd + 2, dint, :],
                            start=(dint == 0), stop=(dint == PO - 1))
                    nc.vector.tensor_add(
                        av_lhs[:, h, qd * 4:(qd + 1) * 4, 0:DH], ps,
                        bv_sb[:, qd * 256:(qd + 1) * 256])

                chains = ([("k", 0)] + [("q", po) for po in range(PO)] +
                          [("k", po) for po in range(1, PO)])
                for i, (which, po) in enumerate(chains):
                    if which == "q":
                        qk_chain(wq_w, xqT, XqT, XqTs, bq_sb, po,
                                 chain_slot(i))
                    else:
                        qk_chain(wk_w, xkT, XkT, XkTs, bk_sb, po,
                                 chain_slot(i))

                # ---- deferred-work queues ----
                av_queue = []     # per-k-group AV closures (order = accum order)
                bg_queue = []     # (cost_ns, is_borrow, closure)

                def drain_av(n=1):
                    for _ in range(n):
                        if av_queue:
                            av_queue.pop(0)()

                def drain_bg(budget):
                    borrowed = False
                    while bg_queue and budget > 0:
                        cost, kind, fn = bg_queue[0]
                        borrow = kind == "v"
                        if borrow and borrowed:
                            break
                        if cost > budget and budget < 1400:
                            break
                        bg_queue.pop(0)
                        fn()
                        budget -= cost
                        borrowed = borrowed or borrow

                # V projection for head h, dout-quarter qd (N=256),
                # borrowing R2 between supersteps
                def v_item(h, qd):
                    def run():
                        v_chunk(h, qd, R2[:, 0:256])
                    return (1400, "v", run)

                for qd in range(4):
                    bg_queue.append(v_item(0, qd))

                finish_state = {}

                # finish items for unit (h, mh): DMA-transpose each
                # [96,128] chunk of oT (bf16) into fin_sb [128, 96], then
                # rcp + mul on DVE. No PE or PSUM involvement at all.
                def fin_item(h, mh, j):
                    def run():
                        oT = finish_state[(h, mh)]
                        fsb = finish_state[(h, mh, "fsb")]
                        stage = finish_state.get((h, "stage"))
                        if stage is None:
                            stage = fin_pool.tile([P, 16, DH], f32,
                                                  tag="stage", bufs=2,
                                                  name=f"stage{h}")
                            finish_state[(h, "stage")] = stage
                        if j == 0:
                            # one whole-unit transpose: [96,1024]->[128,8,96]
                            nc.sync.dma_start(out=fsb[:], in_=oT[:],
                                              transpose=True)
                        rcp = fin_pool.tile([P, 1], f32, tag="rcp")
                        nc.vector.reciprocal(rcp[:], fsb[:, j, DH:DH + 1])
                        nc.vector.tensor_scalar_mul(stage[:, 2 * j + mh, :],
                                                    fsb[:, j, 0:DH], rcp[:])
                    return (150, "fin", run)

                def dma_out_item(h):
                    def run():
                        stage = finish_state.pop((h, "stage"))
                        nc.sync.dma_start(out=out_w[h, :, :, :], in_=stage[:])
                    return (100, "out", run)

                # ---------------- attention units ----------------
                def unit(h, mh, first_unit, last_unit):
                    av_t = avp.tile([DH + 1, 1024], f32, tag="av",
                                    name=f"av{h}_{mh}")
                    qA = XqT if mh == 0 else XqTs
                    qB = XqTs if mh == 0 else XqT

                    def pair(g, pos):
                        kA = XkT if (g % 2) == 0 else XkTs
                        kB = XkTs if (g % 2) == 0 else XkT
                        dst = (R01[:, 0:1024] if pos == 0 else
                               R01[:, 1024:2048] if pos == 1 else R2[:])
                        nc.tensor.matmul(
                            dst[:, 0:512],
                            kA[0:64, g // 2, h * P:(h + 1) * P],
                            qA[0:64, 0:4, h * P:(h + 1) * P],
                            start=True, stop=True)
                        nc.tensor.matmul(
                            dst[:, 512:1024],
                            kB[64:128, g // 2, h * P:(h + 1) * P],
                            qB[64:128, 4:8, h * P:(h + 1) * P],
                            start=True, stop=True)

                    def push_av(g, at_ap, first, last):
                        def run():
                            for half in range(2):
                                nc.tensor.matmul(
                                    av_t[:, half * 512:(half + 1) * 512],
                                    av_lhs[:, h, g, :],
                                    at_ap[:, half * 512:(half + 1) * 512],
                                    start=first, stop=last)
                        av_queue.append(run)

                    # PE order per superstep: pairs FIRST (they only wait
                    # the previous exp's read, so they run while ACT is on
                    # the previous pos2 exp), then deferred AV of the
                    # previous superstep, then bg, then this superstep's
                    # A/B AVs. Keeps ACT gapless.
                    hold = 3 if first_unit else 0
                    for c in range(6):
                        gA, gB, gC = 3 * c, 3 * c + 1, 3 * c + 2
                        if c < 5:
                            pair(gA, 0)
                            pair(gB, 1)
                            at2 = attn_pool.tile([P, 2048], bf16, tag="at2")
                            nc.scalar.activation(at2[:], R01[:], Exp,
                                                 scale=1.0)
                            if c == 0 and finish_state.get("pending"):
                                finish_state.pop("pending")()
                            if c >= hold:
                                drain_av(4 if first_unit else 2)
                            pair(gC, 2)
                            at1 = attn_pool.tile([P, 1024], bf16, tag="at1")
                            nc.scalar.activation(at1[:], R2[:], Exp,
                                                 scale=1.0)
                            if c >= hold:
                                drain_av(2 if first_unit else 1)
                            # PSUM-borrow items (V chains, budget>=1400)
                            # only in early supersteps: a borrow near the
                            # unit end makes the g15 exp hostage to its
                            # DVE evict via the R2 tile WAR.
                            drain_bg((2400 if c < 4 else 900) if first_unit
                                     else (1900 if c in (0, 2) else 1100))
                            push_av(gA, at2[:, 0:1024], gA == 0, False)
                            push_av(gB, at2[:, 1024:2048], False, False)
                            push_av(gC, at1[:], False, gC == KB - 1)
                        else:
                            g15 = 15
                            pair(g15, 2)
                            at1 = attn_pool.tile([P, 1024], bf16, tag="at1")
                            nc.scalar.activation(at1[:], R2[:], Exp,
                                                 scale=1.0)
                            drain_av(3)
                            drain_bg(900)
                            push_av(g15, at1[:], False, True)
                            # g15's AV stays queued: the NEXT unit's pairs
                            # run during exp(g15), closing the boundary gap

                    # deferred finish: drain g15's AV, evict the
                    # accumulator (bf16, rows 0:65 of a 96-row tile so the
                    # DMA-transpose chunk is [96,1024]), queue fin work.
                    # Runs at the START of the next unit so this unit's
                    # last exp overlaps the next unit's first pairs.
                    def finish_unit(h=h, mh=mh, av_t=av_t):
                        finish_body(h, mh, av_t)
                    finish_state["pending"] = finish_unit

                def finish_body(h, mh, av_t):
                    drain_av(len(av_queue))
                    oT = fin_pool.tile([96, 1024], bf16, tag="oT", bufs=3)
                    nc.vector.tensor_copy(oT[0:DH + 1, :], av_t[:])
                    finish_state[(h, mh)] = oT
                    fsb_t = fin_pool.tile([P, PO, 96], bf16, tag="fsb",
                                          bufs=3, name=f"fsb{h}_{mh}")
                    finish_state[(h, mh, "fsb")] = fsb_t
                    if mh == 1:
                        # stage DMA needs ALL 16 fin muls done first: flush
                        # any straggler fins of (h, 0) now, put (h, 1) fins
                        # in front, and the dma_out at the very back.
                        rest = []
                        for item in bg_queue:
                            if item[1] == "fin":
                                item[2]()
                            else:
                                rest.append(item)
                        bg_queue[:] = rest
                    # fins at the BACK: V items (hard deadline) keep
                    # priority; the mh==1 flush above is the correctness
                    # backstop for the stage DMA.
                    bg_queue.extend(fin_item(h, mh, j) for j in range(PO))
                    if mh == 1:
                        bg_queue.append(dma_out_item(h))

                # unit loop with V drip for upcoming heads
                for h in range(HPC):
                    for mh in range(2):
                        if mh == 1 and h + 1 < HPC:
                            # front: next head's AVs need these within ~1 unit
                            bg_queue[0:0] = [v_item(h + 1, qd)
                                             for qd in range(4)]
                        unit(h, mh, first_unit=(h == 0 and mh == 0),
                             last_unit=(h == HPC - 1 and mh == 1))

                # tail: last unit's deferred finish, then remaining work
                if finish_state.get("pending"):
                    finish_state.pop("pending")()
                while bg_queue:
                    _, _, fn = bg_queue.pop(0)
                    fn()

    nc.compile()
    _dedupe_ldweights(nc)
    _BUILT[("nc", reps, salt)] = nc
    return nc


def _dedupe_ldweights(nc):
    """Remove InstLdweights that reload the stationary already resident in
    the PE array (consecutive matmuls sharing lhsT)."""
    def key(a):
        return (str(a.memref), a.offset, str(a.ap), str(a.dtype))

    for f in nc.m.functions:
        for b in f.blocks:
            last = None
            keep = []
            for i in b.instructions:
                tn = type(i).__name__
                if tn == "InstLdweights":
                    k = key(i.ins[0])
                    si = i.sync_info
                    clean = (si is None) or (not si.on_wait and not si.on_update)
                    if last == k and clean:
                        continue
                    last = k
                elif tn == "InstMatmult":
                    if i.is_transpose:
                        last = None
                elif tn in ("InstDrain", "InstUnconditionalBranch", "InstCall"):
                    last = None
                keep.append(i)
            b.instructions[:] = keep


def _make_in_maps(q, k, v, Wq, bq, Wk, bk, Wv, bv):
    import ml_dtypes
    bfl = ml_dtypes.bfloat16

    q = np.asarray(q, dtype=np.float32)
    k = np.asarray(k, dtype=np.float32)
    v = np.asarray(v, dtype=np.float32)

    def slab(w):
        # [din, dout] -> [p, s, t, c] with din = t*128+p, dout = s*128+c
        return np.ascontiguousarray(
            w.reshape(PO, 128, PO, 128).transpose(1, 2, 0, 3))

    Wq_b = slab((np.asarray(Wq, np.float32) * 0.125).astype(bfl))
    Wk_b = slab(np.asarray(Wk, np.float32).astype(bfl))
    Wv_b = slab(np.asarray(Wv, np.float32).astype(bfl))
    bq_t = np.ascontiguousarray(
        (np.asarray(bq, np.float32) * 0.125).reshape(PO, 128).T)
    bk_t = np.ascontiguousarray(np.asarray(bk, np.float32).reshape(PO, 128).T)
    bv_t = np.ascontiguousarray(np.asarray(bv, np.float32).reshape(1, D))

    in_maps = []
    for c in range(NCORES):
        b = c // (NCORES // B)
        r0 = (c % (NCORES // B)) * ROWS
        def xprep(x):
            # [ROWS, D] -> [p, t, m] with din = t*128 + p
            return np.ascontiguousarray(
                x[b, r0:r0 + ROWS, :].T.astype(bfl)
                .reshape(PO, 128, ROWS).transpose(1, 0, 2))
        in_maps.append({
            "qT": xprep(q), "kT": xprep(k), "vT": xprep(v),
            "Wq": Wq_b, "Wk": Wk_b, "Wv": Wv_b,
            "bq": bq_t, "bk": bk_t, "bv": bv_t,
        })
    return in_maps


def kernel(q, k, v, Wq, bq, Wk, bk, Wv, bv):
    from concourse.bass_utils import run_bass_kernel_spmd

    nc = _build_nc()
    in_maps = _make_in_maps(q, k, v, Wq, bq, Wk, bk, Wv, bv)
    res = run_bass_kernel_spmd(nc, in_maps, core_ids=list(range(NCORES)))

    outp = np.empty((B, NH, S, DH), dtype=np.float32)
    for c in range(NCORES):
        b = c // (NCORES // B)
        h0 = (c % (NCORES // B)) * HPC
        outp[b, h0:h0 + HPC] = res.results[c]["out"]
    return outp


# revision 5
# speedup vs baseline: 1.0495x; 1.0079x over previous
"""Multi-head attention (no-transpose head reshape) on 8 trn2 cores — v3.

Sharding: core c gets b=c//4, heads 4*(c%4)..+4 (rows 512*(c%4)..+512 of
batch b). 32 independent (b, h) tasks, no collectives.

v3 vs baseline (219.7us -> ~210us):
  - Row-tiled score pairs: the two N=512 score MMs of each k-group run
    CONCURRENTLY in the PE array (tile_position (0,0)+(64,0), K=64
    each), halving scores PE time. Needs partition-swapped copies of
    both XqT (XqTs) and XkT (XkTs), made by gpsimd SBUF-SBUF DMA.
  - Unit = (head, m-half); m-half outer so ONE [65,1024] AV accumulator
    (2 PSUM banks) is live. Scores live in R01 [128,2048] (4 banks) +
    R2 [128,1024] (2 banks): superstep = 3 k-groups (pos0/pos1 in R01,
    pos2 in R2); exp batched N=2048 over R01, N=1024 over R2. Separate
    tiles matter: Tile tracks cross-engine PSUM deps at TILE
    granularity, so pairC/V-borrows in R2 don't serialize against the
    R01 exp (and vice versa).
  - PE emission order per superstep keeps ACT gapless: pairs first
    (they only wait the previous exp read), then deferred AVs, then bg.
  - Input DMA: host pre-lays x as [p, t, m] and W as [p, s, t, c]
    (fully contiguous per partition -> ~128 descriptors per DMA instead
    of 1024+), weights in per-slab DMAs so projection chains pipeline
    behind the transfers. PE warm-up MMs gated on the xq DMA so HAM
    reaches K=8/8 before the chains.
  - Pre-attention: full Q/K projections as N=512 chains alternating
    R01/R2 slots (tile-granular WAR => no chain-boundary stalls);
    V dripped per head through the bg queue into R2 between supersteps.
  - Finish per unit: ONE whole-unit DMA-transpose (HWDGE xbar)
    [96,1024] bf16 -> [128,8,96], then rcp + per-chunk scalar-mul into
    stage, one 512KB DMA per head. No PE or PSUM involvement.
"""

import numpy as np

B, S, D, NH = 2, 2048, 1024, 16
DH = 64
NCORES = 8
HPC = NH * B // NCORES      # heads per core = 4
ROWS = HPC * (S // NH)      # projection rows per core = 512
PO = D // 128               # 8 din/dout tiles

_BUILT = {}


def _build_nc(reps=1, salt=0.0):
    if ("nc", reps, salt) in _BUILT:
        return _BUILT[("nc", reps, salt)]

    import concourse.bass as bass
    import concourse.bacc as bacc
    import concourse.tile as tile
    from concourse import mybir
    from concourse.masks import make_identity
    from contextlib import ExitStack

    f32 = mybir.dt.float32
    bf16 = mybir.dt.bfloat16
    Exp = mybir.ActivationFunctionType.Exp

    nc = bacc.Bacc("TRN2", target_bir_lowering=False, debug=False)

    # x inputs host-laid-out as [p, t, m] (contiguous 8KB/partition)
    qT = nc.dram_tensor("qT", [128, PO, ROWS], bf16, kind="ExternalInput")
    kT = nc.dram_tensor("kT", [128, PO, ROWS], bf16, kind="ExternalInput")
    vT = nc.dram_tensor("vT", [128, PO, ROWS], bf16, kind="ExternalInput")
    # partition-major: [p, s, t, c] = W[t*128+p, s*128+c] — 16KB
    # contiguous per partition so one DMA with minimal descriptors moves
    # a whole weight matrix.
    Wq = nc.dram_tensor("Wq", [128, PO, PO, 128], bf16, kind="ExternalInput")
    Wk = nc.dram_tensor("Wk", [128, PO, PO, 128], bf16, kind="ExternalInput")
    Wv = nc.dram_tensor("Wv", [128, PO, PO, 128], bf16, kind="ExternalInput")
    bq = nc.dram_tensor("bq", [128, PO], f32, kind="ExternalInput")
    bk = nc.dram_tensor("bk", [128, PO], f32, kind="ExternalInput")
    bv = nc.dram_tensor("bv", [1, D], f32, kind="ExternalInput")
    out = nc.dram_tensor("out", [HPC, S, DH], f32, kind="ExternalOutput")
    out_w = out.ap().rearrange("h (a r) d -> h a r d", r=16)

    P = 128
    KB = S // P                # 16 k-groups per head

    with tile.TileContext(nc) as tc, ExitStack() as ctx:
        consts = ctx.enter_context(tc.tile_pool(name="consts", bufs=1))
        wrows = ctx.enter_context(tc.tile_pool(name="wrows", bufs=1))
        inputs = ctx.enter_context(tc.tile_pool(name="inputs", bufs=1))
        proj = ctx.enter_context(tc.tile_pool(name="proj", bufs=1))

        for _rep in range(reps):
            xqT = inputs.tile([P, PO, ROWS], bf16, tag="xqT")
            xkT = inputs.tile([P, PO, ROWS], bf16, tag="xkT")
            xvT = inputs.tile([P, PO, ROWS], bf16, tag="xvT")
            XqT = proj.tile([P, PO, ROWS], bf16, tag="XqT")
            XqTs = proj.tile([P, PO, ROWS], bf16, tag="XqTs")
            XkT = proj.tile([P, PO, ROWS], bf16, tag="XkT")
            XkTs = proj.tile([P, PO, ROWS], bf16, tag="XkTs")
            av_lhs = proj.tile([P, HPC, KB, DH + 1], bf16, tag="av_lhs")

            # weight layout [p, s(dout-slab), t(din-tile), c]: lhsT for
            # (dint, po) = w[:, po, dint, :]
            wq_w = wrows.tile([P, PO, PO, P], bf16, tag="wq")
            wk_w = wrows.tile([P, PO, PO, P], bf16, tag="wk")
            wv_w = wrows.tile([P, PO, PO, P], bf16, tag="wv")

            bq_sb = consts.tile([P, PO], f32, tag="bq")
            bk_sb = consts.tile([P, PO], f32, tag="bk")
            bv_sb = consts.tile([P, D], f32, tag="bv")
            identf = consts.tile([P, P], f32, tag="identf")

            # ---------------- input DMAs (deadline order) ----------------
            xqsrc = qT.ap()
            xksrc = kT.ap()
            xvsrc = vT.ap()

            # sync: Q path (attention-start critical); per-slab weight
            # DMAs (contiguous (t,c) 2KB per partition -> cheap triggers)
            # so projection chains pipeline behind the transfers.
            # exp table load first, before any scalar-queue DMA triggers
            warm = consts.tile([1, 1], f32, tag="warm")
            nc.vector.memset(warm[:], salt)
            nc.scalar.activation(warm[:], warm[:], Exp, scale=1.0)
            nc.sync.dma_start(out=xqT[:], in_=xqsrc[:])
            for s in range(PO):
                nc.sync.dma_start(out=wq_w[:, s], in_=Wq.ap()[:, s])
            # scalar: K path, then V path
            nc.scalar.dma_start(out=xkT[:], in_=xksrc[:])
            for s in range(PO):
                nc.scalar.dma_start(out=wk_w[:, s], in_=Wk.ap()[:, s])
            nc.scalar.dma_start(out=xvT[:], in_=xvsrc[:])
            nc.scalar.dma_start(out=wv_w[:, 0:4], in_=Wv.ap()[:, 0:4])
            nc.scalar.dma_start(out=wv_w[:, 4:8], in_=Wv.ap()[:, 4:8])
            # gpsimd: bias broadcast (+ swap DMAs emitted inline later)
            bv_ap = bv.ap()
            bv_bcast = bass.AP(tensor=bv_ap.tensor, offset=bv_ap.offset,
                               ap=[[0, P], [1, D]])
            nc.gpsimd.dma_start(out=bq_sb[:], in_=bq.ap())
            nc.gpsimd.dma_start(out=bk_sb[:], in_=bk.ap())
            nc.gpsimd.dma_start(out=bv_sb[:], in_=bv_bcast)

            make_identity(nc, identf[:])
            nc.vector.memset(av_lhs[:, :, :, DH:DH + 1], 1.0)

            # ---------------- PSUM pools ----------------
            with tc.tile_pool(name="Rp", bufs=1, space="PSUM") as Rp, \
                 tc.tile_pool(name="avp", bufs=1, space="PSUM") as avp, \
                 tc.tile_pool(name="attn", bufs=6) as attn_pool, \
                 tc.tile_pool(name="fin", bufs=4) as fin_pool:

                # Two score tiles: cross-engine deps are tracked at TILE
                # granularity, so the batched exp reads exactly R01 and
                # pos2 work (pairC, V/tp borrows) lives in R2.
                R01 = Rp.tile([P, 2048], f32, tag="R01")
                R2 = Rp.tile([P, 1024], f32, tag="R2")

                # ---- pre-attention: FULL Q/K projections (N=512 chains,
                # one per po), cycling R slots of 512 cols. K po0 first
                # (first scores pair needs it + its swap), then Q (all po
                # gate the first pair's rhs), then K rest.
                def qk_chain(w_sb, x_sb, X_dst, X_swp, b_sb, po, ps):
                    for dint in range(PO):
                        nc.tensor.matmul(
                            ps, w_sb[:, po, dint, :],
                            x_sb[:, dint, :],
                            start=(dint == 0), stop=(dint == PO - 1))
                    nc.vector.tensor_scalar_add(X_dst[:, po, :], ps,
                                                b_sb[:, po:po + 1])
                    nc.gpsimd.dma_start(out=X_swp[0:64, po, :],
                                        in_=X_dst[64:128, po, :])
                    nc.gpsimd.dma_start(out=X_swp[64:128, po, :],
                                        in_=X_dst[0:64, po, :])

                # PE warm-up: ~1.3us of tiny matmuls gated on the same
                # deps as chain 0 (xq + Wq slab 0) so HAM reaches K=8/8
                # right as the real chains start.
                for dint in range(PO):
                    nc.tensor.matmul(R01[:, 0:P], xqT[:, dint, 0:P],
                                     xqT[:, dint, 0:P],
                                     start=True, stop=True)

                def chain_slot(i):
                    # alternate tiles so chain i+1 never waits chain i's
                    # eviction (tile-granular WAR)
                    if i % 2 == 0:
                        return R01[:, ((i // 2) % 4) * 512:
                                   ((i // 2) % 4) * 512 + 512]
                    return R2[:, ((i // 2) % 2) * 512:
                              ((i // 2) % 2) * 512 + 512]

                def v_chunk(h, qd, ps):
                    for dint in range(PO):
                        nc.tensor.matmul(
                            ps, xvT[:, dint, h * P:(h + 1) * P],
                            wv_w[:, 2 * qd:2 * qd + 2, dint, :],
                            start=(dint == 0), stop=(dint == PO - 1))
                    nc.vector.tensor_add(
                        av_lhs[:, h, qd * 4:(qd + 1) * 4, 0:DH], ps,
                        bv_sb[:, qd * 256:(qd + 1) * 256])

                chains = ([("k", 0)] + [("q", po) for po in range(PO)] +
                          [("k", po) for po in range(1, PO)])
                for i, (which, po) in enumerate(chains):
                    if which == "q":
                        qk_chain(wq_w, xqT, XqT, XqTs, bq_sb, po,
                                 chain_slot(i))
                    else:
                        qk_chain(wk_w, xkT, XkT, XkTs, bk_sb, po,
                                 chain_slot(i))

                # ---- deferred-work queues ----
                av_queue = []     # per-k-group AV closures (order = accum order)
                bg_queue = []     # (cost_ns, is_borrow, closure)

                def drain_av(n=1):
                    for _ in range(n):
                        if av_queue:
                            av_queue.pop(0)()

                def drain_bg(budget):
                    borrowed = False
                    while bg_queue and budget > 0:
                        cost, kind, fn = bg_queue[0]
                        borrow = kind == "v"
                        if borrow and borrowed:
                            break
                        if cost > budget and budget < 1400:
                            break
                        bg_queue.pop(0)
                        fn()
                        budget -= cost
                        borrowed = borrowed or borrow

                # V projection for head h, dout-quarter qd (N=256),
                # borrowing R2 between supersteps
                def v_item(h, qd):
                    def run():
                        v_chunk(h, qd, R2[:, 0:256])
                    return (1400, "v", run)

                for qd in range(4):
                    bg_queue.append(v_item(0, qd))

                finish_state = {}

                # finish items for unit (h, mh): DMA-transpose each
                # [96,128] chunk of oT (bf16) into fin_sb [128, 96], then
                # rcp + mul on DVE. No PE or PSUM involvement at all.
                def fin_item(h, mh, j):
                    def run():
                        oT = finish_state[(h, mh)]
                        fsb = finish_state[(h, mh, "fsb")]
                        stage = finish_state.get((h, "stage"))
                        if stage is None:
                            stage = fin_pool.tile([P, 16, DH], f32,
                                                  tag="stage", bufs=2,
                                                  name=f"stage{h}")
                            finish_state[(h, "stage")] = stage
                        if j == 0:
                            # one whole-unit transpose: [96,1024]->[128,8,96]
                            nc.sync.dma_start(out=fsb[:], in_=oT[:],
                                              transpose=True)
                        rcp = fin_pool.tile([P, 1], f32, tag="rcp")
                        nc.vector.reciprocal(rcp[:], fsb[:, j, DH:DH + 1])
                        nc.vector.tensor_scalar_mul(stage[:, 2 * j + mh, :],
                                                    fsb[:, j, 0:DH], rcp[:])
                    return (150, "fin", run)

                def dma_out_item(h):
                    def run():
                        stage = finish_state.pop((h, "stage"))
                        nc.sync.dma_start(out=out_w[h, :, :, :], in_=stage[:])
                    return (100, "out", run)

                # ---------------- attention units ----------------
                def unit(h, mh, first_unit, last_unit):
                    av_t = avp.tile([DH + 1, 1024], f32, tag="av",
                                    name=f"av{h}_{mh}")
                    qA = XqT if mh == 0 else XqTs
                    qB = XqTs if mh == 0 else XqT

                    def pair(g, pos):
                        kA = XkT if (g % 2) == 0 else XkTs
                        kB = XkTs if (g % 2) == 0 else XkT
                        dst = (R01[:, 0:1024] if pos == 0 else
                               R01[:, 1024:2048] if pos == 1 else R2[:])
                        nc.tensor.matmul(
                            dst[:, 0:512],
                            kA[0:64, g // 2, h * P:(h + 1) * P],
                            qA[0:64, 0:4, h * P:(h + 1) * P],
                            start=True, stop=True)
                        nc.tensor.matmul(
                            dst[:, 512:1024],
                            kB[64:128, g // 2, h * P:(h + 1) * P],
                            qB[64:128, 4:8, h * P:(h + 1) * P],
                            start=True, stop=True)

                    def push_av(g, at_ap, first, last):
                        def run():
                            for half in range(2):
                                nc.tensor.matmul(
                                    av_t[:, half * 512:(half + 1) * 512],
                                    av_lhs[:, h, g, :],
                                    at_ap[:, half * 512:(half + 1) * 512],
                                    start=first, stop=last)
                        av_queue.append(run)

                    # PE order per superstep: pairs FIRST (they only wait
                    # the previous exp's read, so they run while ACT is on
                    # the previous pos2 exp), then deferred AV of the
                    # previous superstep, then bg, then this superstep's
                    # A/B AVs. Keeps ACT gapless.
                    hold = 3 if first_unit else 0
                    for c in range(6):
                        gA, gB, gC = 3 * c, 3 * c + 1, 3 * c + 2
                        if c < 5:
                            pair(gA, 0)
                            pair(gB, 1)
                            at2 = attn_pool.tile([P, 2048], bf16, tag="at2")
                            nc.scalar.activation(at2[:], R01[:], Exp,
                                                 scale=1.0)
                            if c == 0 and finish_state.get("pending"):
                                finish_state.pop("pending")()
                            if c >= hold:
                                drain_av(4 if first_unit else 2)
                            pair(gC, 2)
                            at1 = attn_pool.tile([P, 1024], bf16, tag="at1")
                            nc.scalar.activation(at1[:], R2[:], Exp,
                                                 scale=1.0)
                            if c >= hold:
                                drain_av(2 if first_unit else 1)
                            # PSUM-borrow items (V chains, budget>=1400)
                            # only in early supersteps: a borrow near the
                            # unit end makes the g15 exp hostage to its
                            # DVE evict via the R2 tile WAR.
                            drain_bg((2400 if c < 4 else 900) if first_unit
                                     else (1900 if c in (0, 2) else 1100))
                            push_av(gA, at2[:, 0:1024], gA == 0, False)
                            push_av(gB, at2[:, 1024:2048], False, False)
                            push_av(gC, at1[:], False, gC == KB - 1)
                        else:
                            g15 = 15
                            pair(g15, 2)
                            at1 = attn_pool.tile([P, 1024], bf16, tag="at1")
                            nc.scalar.activation(at1[:], R2[:], Exp,
                                                 scale=1.0)
                            drain_av(2)
                            drain_bg(900)
                            push_av(g15, at1[:], False, True)
                            # g15's AV stays queued: the NEXT unit's pairs
                            # run during exp(g15), closing the boundary gap

                    # deferred finish: drain g15's AV, evict the
                    # accumulator (bf16, rows 0:65 of a 96-row tile so the
                    # DMA-transpose chunk is [96,1024]), queue fin work.
                    # Runs at the START of the next unit so this unit's
                    # last exp overlaps the next unit's first pairs.
                    def finish_unit(h=h, mh=mh, av_t=av_t):
                        finish_body(h, mh, av_t)
                    finish_state["pending"] = finish_unit

                def finish_body(h, mh, av_t):
                    drain_av(len(av_queue))
                    oT = fin_pool.tile([96, 1024], bf16, tag="oT", bufs=3)
                    nc.vector.tensor_copy(oT[0:DH + 1, :], av_t[:])
                    finish_state[(h, mh)] = oT
                    fsb_t = fin_pool.tile([P, PO, 96], bf16, tag="fsb",
                                          bufs=3, name=f"fsb{h}_{mh}")
                    finish_state[(h, mh, "fsb")] = fsb_t
                    if mh == 1:
                        # stage DMA needs ALL 16 fin muls done first: flush
                        # any straggler fins of (h, 0) now, put (h, 1) fins
                        # in front, and the dma_out at the very back.
                        rest = []
                        for item in bg_queue:
                            if item[1] == "fin":
                                item[2]()
                            else:
                                rest.append(item)
                        bg_queue[:] = rest
                    # fins at the BACK: V items (hard deadline) keep
                    # priority; the mh==1 flush above is the correctness
                    # backstop for the stage DMA.
                    bg_queue.extend(fin_item(h, mh, j) for j in range(PO))
                    if mh == 1:
                        bg_queue.append(dma_out_item(h))

                # unit loop with V drip for upcoming heads
                for h in range(HPC):
                    for mh in range(2):
                        if mh == 1 and h + 1 < HPC:
                            # front: next head's AVs need these within ~1 unit
                            bg_queue[0:0] = [v_item(h + 1, qd)
                                             for qd in range(4)]
                        unit(h, mh, first_unit=(h == 0 and mh == 0),
                             last_unit=(h == HPC - 1 and mh == 1))

                # tail: last unit's deferred finish, then remaining work
                if finish_state.get("pending"):
                    finish_state.pop("pending")()
                while bg_queue:
                    _, _, fn = bg_queue.pop(0)
                    fn()

    nc.compile()
    _dedupe_ldweights(nc)
    _BUILT[("nc", reps, salt)] = nc
    return nc


def _dedupe_ldweights(nc):
    """Remove InstLdweights that reload the stationary already resident in
    the PE array (consecutive matmuls sharing lhsT)."""
    def key(a):
        return (str(a.memref), a.offset, str(a.ap), str(a.dtype))

    for f in nc.m.functions:
        for b in f.blocks:
            last = None
            keep = []
            for i in b.instructions:
                tn = type(i).__name__
                if tn == "InstLdweights":
                    k = key(i.ins[0])
                    si = i.sync_info
                    clean = (si is None) or (not si.on_wait and not si.on_update)
                    if last == k and clean:
                        continue
                    last = k
                elif tn == "InstMatmult":
                    if i.is_transpose:
                        last = None
                elif tn in ("InstDrain", "InstUnconditionalBranch", "InstCall"):
                    last = None
                keep.append(i)
            b.instructions[:] = keep


def _make_in_maps(q, k, v, Wq, bq, Wk, bk, Wv, bv):
    import ml_dtypes
    bfl = ml_dtypes.bfloat16

    q = np.asarray(q, dtype=np.float32)
    k = np.asarray(k, dtype=np.float32)
    v = np.asarray(v, dtype=np.float32)

    def slab(w):
        # [din, dout] -> [p, s, t, c] with din = t*128+p, dout = s*128+c
        return np.ascontiguousarray(
            w.reshape(PO, 128, PO, 128).transpose(1, 2, 0, 3))

    Wq_b = slab((np.asarray(Wq, np.float32) * 0.125).astype(bfl))
    Wk_b = slab(np.asarray(Wk, np.float32).astype(bfl))
    Wv_b = slab(np.asarray(Wv, np.float32).astype(bfl))
    bq_t = np.ascontiguousarray(
        (np.asarray(bq, np.float32) * 0.125).reshape(PO, 128).T)
    bk_t = np.ascontiguousarray(np.asarray(bk, np.float32).reshape(PO, 128).T)
    bv_t = np.ascontiguousarray(np.asarray(bv, np.float32).reshape(1, D))

    in_maps = []
    for c in range(NCORES):
        b = c // (NCORES // B)
        r0 = (c % (NCORES // B)) * ROWS
        def xprep(x):
            # [ROWS, D] -> [p, t, m] with din = t*128 + p
            return np.ascontiguousarray(
                x[b, r0:r0 + ROWS, :].T.astype(bfl)
                .reshape(PO, 128, ROWS).transpose(1, 0, 2))
        in_maps.append({
            "qT": xprep(q), "kT": xprep(k), "vT": xprep(v),
            "Wq": Wq_b, "Wk": Wk_b, "Wv": Wv_b,
            "bq": bq_t, "bk": bk_t, "bv": bv_t,
        })
    return in_maps


def kernel(q, k, v, Wq, bq, Wk, bk, Wv, bv):
    from concourse.bass_utils import run_bass_kernel_spmd

    nc = _build_nc()
    in_maps = _make_in_maps(q, k, v, Wq, bq, Wk, bk, Wv, bv)
    res = run_bass_kernel_spmd(nc, in_maps, core_ids=list(range(NCORES)))

    outp = np.empty((B, NH, S, DH), dtype=np.float32)
    for c in range(NCORES):
        b = c // (NCORES // B)
        h0 = (c % (NCORES // B)) * HPC
        outp[b, h0:h0 + HPC] = res.results[c]["out"]
    return outp
